# revision 13
# baseline (speedup 1.0000x reference)
"""Trainium2 Bass kernel for nn_MessageGeneratorRNN (fp8 DoubleRow version).

Math (per batch row n, per step t):
    h = tanh(W_ih @ e + b_ih + W_hh @ h_prev + b_hh)
    z = W_out @ h + b_out + g_t
    x = softmax(z)                      -> output slice  [N, NOS, VOCAB]
    e = W_emb @ x + b_emb

Strategy (evolved from the bf16 baseline, 607us -> target ~150us):
  - Data-parallel over N = 4096: 512 rows per core, 8 cores, no collectives.
  - All four weight matmuls run as fp8e4 (e4m3) matmuls in DoubleRow perf
    mode: operands are packed [128, 2, *] so each instruction contracts
    K=256 at 0.5 cycles/row -- 4x the bf16 matmul rate on the PE.
  - Weights are pre-scaled by 32 on the host before fp8 quantization
    (entries ~U(-1/32,1/32) would land in e4m3's subnormal range); the 1/32
    dequant folds into the activations' `scale` / the e-STT scalar.
  - Softmax is per-column scale-invariant, so the host subtracts the
    per-column gumbel max (g' = g - max_j g_j) which bounds u = exp(z+g')
    <= e^3 << 240 (fp8 max).  u is produced in bf16 (accuracy for the
    output and implicitly the softmax sum); a DVE copy makes the fp8
    packed version used by the W_emb matmul and the on-device sum.
  - The on-device sum s (fp8) only normalizes e -- tolerant of fp8 error.
    The OUTPUT normalization happens on the host: x = u_bf16 / sum(u_bf16),
    so output accuracy is bf16-grade (sim: rel_err ~3e-3 vs 2e-2 budget).
  - g' ships as fp16 (error ~ |g'| ulp; large-|g'| entries are
    exponentially suppressed in the softmax so this is safe) and is added
    on the Pool engine: acc = acc*(1/32) + g (frees DVE).
  - One batched DMA per step for g-in and u-out (128x8x512 tiles) keeps
    the SP sequencer + HWDGE off the critical path.
"""

import os
import sys

import numpy as np

for _p in ("/root/.axon_site/_ro/trn_rl_repo", "/opt/trn_rl_repo"):
    if _p not in sys.path and os.path.isdir(_p):
        sys.path.append(_p)

import concourse.bass as bass
import concourse.mybir as mybir
import concourse.tile as tile
from concourse.alu_op_type import AluOpType
from concourse.bass_utils import run_bass_kernel_spmd

VOCAB = 1024
HID = 1024
EMB = 256
NOS = 12
N = 4096
NCORES = 8
NS = N // NCORES          # 512 rows per core
P = 128                   # partitions
KH = HID // P             # 8 hid tiles of 128
KV = VOCAB // P           # 8 vocab tiles
KP = HID // (2 * P)       # 4 double-row pairs over hid/vocab
WS = 32.0                 # weight pre-scale (dequant via act scale)

F8 = mybir.dt.float8e4
BF16 = mybir.dt.bfloat16
F16 = mybir.dt.float16
F32 = mybir.dt.float32
F32R = mybir.dt.float32r
F8NP = mybir.dt.np(F8)
BF16NP = mybir.dt.np(BF16)
ACT = mybir.ActivationFunctionType
DR = mybir.MatmulPerfMode.DoubleRow


# ---------------------------------------------------------------------------
# Workaround: this walrus build supports only ONE sem wait per instruction
# ("Too many sync wait commands"), while Tile emits multi-wait instructions
# routinely.  Post-pass: move all but the last wait of every instruction onto
# fresh same-engine NoOps inserted immediately before it (same-engine program
# order makes this equivalent).
# ---------------------------------------------------------------------------
import bass_rust as _bass_rust


def split_multi_waits(nc):
    ctr = 0
    for f in nc.m.functions:
        for bb in f.blocks:
            new = []
            changed = False
            for inst in list(bb.instructions):
                si = inst.sync_info
                waits = list(si.on_wait) if si is not None else []
                if len(waits) > 1:
                    changed = True
                    for w in waits[:-1]:
                        nop = _bass_rust.InstNoOp(
                            name=f"I-wsplit-{ctr}", engine=inst.engine
                        )
                        ctr += 1
                        nop.sync_info = mybir.SyncInfo(on_wait=[w], on_update=[])
                        new.append(nop)
                    inst.sync_info = mybir.SyncInfo(
                        on_wait=[waits[-1]], on_update=list(si.on_update)
                    )
                new.append(inst)
            if changed:
                bb.instructions = new
    return ctr


# ---------------------------------------------------------------------------
# Device program (identical on every core; SPMD over the batch axis)
# ---------------------------------------------------------------------------
def make_io(nc):
    def dram(name, shape, dt, kind="ExternalInput"):
        return nc.dram_tensor(name, shape, dt, kind=kind).ap()

    return {
        "h0p": dram("h0p", [KP, P, 2, NS], F8),
        "whhp": dram("whhp", [KP, P, 2, HID], F8),
        "woutp": dram("woutp", [KP, P, 2, VOCAB], F8),
        "wihp": dram("wihp", [P, 2, HID], F8),
        "wembp": dram("wembp", [KP, P, 2, EMB], F8),
        "bhp": dram("bhp", [P, 2 * KH], F32),
        "bop": dram("bop", [P, KV], F32),
        "sosp": dram("sosp", [P, 2], F32),
        "gT": dram("gT", [NOS, VOCAB, NS], F16),
        "uout": dram("uout", [NOS, VOCAB, NS], BF16, kind="ExternalOutput"),
    }


class NSpace:
    pass


def emit_setup(tc, io, ctx):
    """Load weights/constants into persistent SBUF tiles."""
    nc = tc.nc
    singles = ctx.enter_context(tc.tile_pool(name="singles", bufs=1))
    sb = NSpace()

    def load(src, shape, dt, tag):
        t = singles.tile(shape, dt, tag=tag, name=tag)
        nc.sync.dma_start(out=t, in_=src)
        return t

    sb.whh = [load(io["whhp"][kp], [P, 2, HID], F8, f"whh{kp}") for kp in range(KP)]
    sb.h0 = [load(io["h0p"][kp], [P, 2, NS], F8, f"h0_{kp}") for kp in range(KP)]
    sb.wih = load(io["wihp"], [P, 2, HID], F8, "wih")
    sb.wout = [load(io["woutp"][kp], [P, 2, VOCAB], F8, f"wout{kp}") for kp in range(KP)]
    sb.wemb = [load(io["wembp"][kp], [P, 2, EMB], F8, f"wemb{kp}") for kp in range(KP)]
    sb.bh = load(io["bhp"], [P, 2 * KH], F32, "bh")
    sb.bo = load(io["bop"], [P, KV], F32, "bo")
    sb.sos = load(io["sosp"], [P, 2], F32, "sos")

    sb.ones_col = singles.tile([P, 1], BF16, tag="ones_col", name="ones_col")
    nc.vector.memset(sb.ones_col, 1.0)
    ones_row_f = singles.tile([1, P], F32, tag="ones_row_f", name="ones_row_f")
    nc.vector.memset(ones_row_f, 1.0)
    sb.ones_row = singles.tile([1, P], F32R, tag="ones_row", name="ones_row")
    with nc.allow_low_precision(reason="bit-copy of exact 1.0s to f32r"):
        nc.vector.tensor_copy(sb.ones_row, ones_row_f)
    sb.ones_blk = singles.tile([P, NS], F32, tag="ones_blk", name="ones_blk")
    nc.vector.memset(sb.ones_blk, 1.0)
    return sb


def make_pools(tc, ctx):
    pl = NSpace()
    pl.h = ctx.enter_context(tc.tile_pool(name="h", bufs=2))
    pl.e = ctx.enter_context(tc.tile_pool(name="e", bufs=2))
    pl.u = ctx.enter_context(tc.tile_pool(name="u", bufs=3))
    pl.u8 = ctx.enter_context(tc.tile_pool(name="u8", bufs=2))
    pl.g = ctx.enter_context(tc.tile_pool(name="g", bufs=3))
    pl.rs = ctx.enter_context(tc.tile_pool(name="rs", bufs=2))
    # PSUM budget (8 banks): h gets 4 so next-step W_hh groups can pre-run
    # during the softmax tail; e-acc reuses the z pool (z-phase is done by
    # the time e-acc runs).
    pl.ps_h = ctx.enter_context(tc.tile_pool(name="ps_h", bufs=4, space="PSUM"))
    pl.ps_z = ctx.enter_context(tc.tile_pool(name="ps_z", bufs=2, space="PSUM"))
    pl.ps_s = ctx.enter_context(tc.tile_pool(name="ps_s", bufs=1, space="PSUM"))
    pl.ps_b = ctx.enter_context(tc.tile_pool(name="ps_b", bufs=1, space="PSUM"))
    return pl


def emit_steps(tc, io, sb, pl):
    """h0/e0 init + the 12-step scan (per-core shard)."""
    nc = tc.nc
    gT, uout = io["gT"], io["uout"]

    # initial state: h0 is preloaded (singles); e0 = sos broadcast along batch
    h_prev = sb.h0
    e_prev = pl.e.tile([P, 2, NS], F8, tag="e", name="e0")
    for i in range(2):
        nc.scalar.activation(
            e_prev[:, i, :], sb.ones_blk, ACT.Copy, scale=sb.sos[:, i:i + 1]
        )

    for t in range(NOS):
        bcol = 1 if t else 0

        # prefetch this step's gumbels (one batched DMA; bufs=3 -> SP runs
        # ahead up to 3 steps)
        gt = pl.g.tile([P, KV, NS], F16, tag="g", name="g")
        nc.sync.dma_start(
            out=gt,
            in_=gT[t].rearrange("(k p) n -> p k n", p=P),
        )

        # ---- h = tanh((W_hh h_prev + W_ih e_prev)/32 + bh) -> fp8 packed ----
        # Emitted in half-batches of 4 m-tiles: the W_hh parts (which don't
        # need e_prev) go first so the PE fills the previous step's softmax
        # tail; the e-dependent W_ih group-closers follow.
        h_new = [pl.h.tile([P, 2, NS], F8, tag=f"h{j}", name=f"h{j}") for j in range(KP)]
        for half in range(2):
            ms = range(half * 4, half * 4 + 4)
            accs = {}
            for m in ms:
                acc = accs[m] = pl.ps_h.tile([P, NS], F32, tag="ps_h", name="ps_h")
                for kp in range(KP):
                    nc.tensor.matmul(
                        acc, lhsT=sb.whh[kp][:, :, m * P:(m + 1) * P], rhs=h_prev[kp],
                        start=(kp == 0), stop=False, perf_mode=DR,
                    )
            for m in ms:
                nc.tensor.matmul(
                    accs[m], lhsT=sb.wih[:, :, m * P:(m + 1) * P], rhs=e_prev,
                    start=False, stop=True, perf_mode=DR,
                )
                nc.scalar.activation(
                    h_new[m // 2][:, m % 2, :], accs[m], ACT.Tanh,
                    bias=sb.bh[:, 2 * m + bcol:2 * m + bcol + 1], scale=1.0 / WS,
                )

        # ---- u = exp((W_out h)/32 + bo) * exp(g)  (gumbel folded as a
        #      post-exp 16-bit SBUF multiply: frees the PSUM bank after the
        #      exp and runs on the DVE 2x path).  The softmax sum s
        #      accumulates from the bf16 u tiles, interleaved into the
        #      z-phase so the tail doesn't wait on the fp8 copies. ----
        ut = pl.u.tile([P, KV, NS], BF16, tag="u", name="u")
        u8 = [pl.u8.tile([P, 2, NS], F8, tag=f"u8{j}", name=f"u8{j}") for j in range(KP)]
        s_ps = pl.ps_s.tile([1, NS], F32, tag="s", name="s")

        def s_partial(m):
            nc.tensor.matmul(
                s_ps, lhsT=sb.ones_col, rhs=ut[:, m, :],
                start=(m == 0), stop=(m == KV - 1),
            )

        for m in range(KV):
            acc = pl.ps_z.tile([P, NS], F32, tag="ps_z", name="ps_z")
            for kp in range(KP):
                nc.tensor.matmul(
                    acc, lhsT=sb.wout[kp][:, :, m * P:(m + 1) * P], rhs=h_new[kp],
                    start=(kp == 0), stop=(kp == KP - 1), perf_mode=DR,
                )
            if m >= 2:
                s_partial(m - 2)  # lags the DVE multiply by ~2 tiles
            nc.scalar.activation(
                ut[:, m, :], acc, ACT.Exp, bias=sb.bo[:, m:m + 1], scale=1.0 / WS
            )
            nc.vector.tensor_tensor(
                ut[:, m, :], ut[:, m, :], gt[:, m, :], op=AluOpType.mult
            )
            with nc.allow_low_precision(reason="fp8 copy of u for e-matmul"):
                eng = nc.vector if m >= KV - 2 else nc.gpsimd
                eng.tensor_copy(u8[m // 2][:, m % 2, :], ut[:, m, :])
        for m in range(KV - 2, KV):
            s_partial(m)
        nc.sync.dma_start(
            out=uout[t].rearrange("(k p) n -> p k n", p=P), in_=ut
        )

        # ---- rs = 1/s ; e-acc = W_emb u8 (doesn't need rs) ; bc ; e-STT ----
        rs = pl.rs.tile([1, NS], F32R, tag="rs", name="rs")
        with nc.allow_low_precision(reason="f32r reciprocal of s"):
            nc.vector.reciprocal(rs, s_ps)
        e_new = pl.e.tile([P, 2, NS], F8, tag="e", name="e")
        e_acc = []
        for me in range(2):
            acc = pl.ps_z.tile([P, NS], F32, tag="ps_z", name="ps_e")
            for kp in range(KP):
                nc.tensor.matmul(
                    acc, lhsT=sb.wemb[kp][:, :, me * P:(me + 1) * P], rhs=u8[kp],
                    start=(kp == 0), stop=(kp == KP - 1), perf_mode=DR,
                )
            e_acc.append(acc)
        b_ps = pl.ps_b.tile([P, NS], F32, tag="bc", name="bc")
        with nc.allow_low_precision(reason="f32r rank-1 broadcast of 1/s"):
            nc.tensor.matmul(b_ps, lhsT=sb.ones_row, rhs=rs, start=True, stop=True)
        # STT may read only one PSUM operand; stage the broadcast in SBUF
        # (Act is idle during this tail). bf16 is plenty for the e scale.
        bc_sb = pl.rs.tile([P, NS], BF16, tag="bc_sb", name="bc_sb")
        nc.scalar.activation(bc_sb, b_ps, ACT.Copy)
        for me in range(2):
            with nc.allow_low_precision(reason="fp8 e scaled by 1/s"):
                nc.vector.scalar_tensor_tensor(
                    out=e_new[:, me, :], in0=e_acc[me], scalar=1.0 / WS, in1=bc_sb,
                    op0=AluOpType.mult, op1=AluOpType.mult,
                )

        h_prev, e_prev = h_new, e_new


def emit_body(tc, io):
    import contextlib

    with contextlib.ExitStack() as ctx:
        sb = emit_setup(tc, io, ctx)
        pl = make_pools(tc, ctx)
        emit_steps(tc, io, sb, pl)


# ---------------------------------------------------------------------------
# Graph construction
# ---------------------------------------------------------------------------
def build_nc(reps=1):
    nc = bass.Bass("TRN2", target_bir_lowering=False, debug=False,
                   num_devices=NCORES)
    io = make_io(nc)
    with tile.TileContext(nc) as tc:
        for _ in range(reps):
            emit_body(tc, io)
    n = split_multi_waits(nc)
    print(f"split_multi_waits: {n} nops inserted")
    return nc


# ---------------------------------------------------------------------------
# Host side: preprocess -> SPMD run -> gather
# ---------------------------------------------------------------------------
def _pack_k(a):
    """[K, M] -> [K/256, 128, 2, M] double-row packing of the contraction."""
    k, m = a.shape
    return np.ascontiguousarray(
        a.reshape(k // 256, 2, P, m).transpose(0, 2, 1, 3)
    )


def make_in_maps(target, gumbels, sos, W_ih, b_ih, W_hh, b_hh, W_out, b_out,
                 W_emb, b_emb):
    f32 = np.float32
    target = np.asarray(target, f32).reshape(N, HID)
    gumbels = np.asarray(gumbels, f32)
    W_ih = np.asarray(W_ih, f32)
    W_hh = np.asarray(W_hh, f32)
    W_out = np.asarray(W_out, f32)
    W_emb = np.asarray(W_emb, f32)
    b_ih = np.asarray(b_ih, f32)
    b_hh = np.asarray(b_hh, f32)
    b_out = np.asarray(b_out, f32)
    b_emb = np.asarray(b_emb, f32)
    sos = np.asarray(sos, f32)

    # h0: [HID, N] double-row packed, fp8
    h0p = _pack_k(target.T).astype(F8NP)                      # [KP,P,2,N]
    # eg = exp(g - colmax) (softmax scale-invariance), fp16, transposed;
    # applied on-device as a post-exp multiply
    g = gumbels - gumbels.max(axis=2, keepdims=True)
    gT = np.ascontiguousarray(np.exp(g).transpose(0, 2, 1)).astype(np.float16)

    bh0 = b_ih + b_hh
    bh1 = bh0 + W_ih @ b_emb
    bh = np.stack([bh0, bh1], axis=1).reshape(KH, P, 2)       # [m][p][col]
    bhp = np.ascontiguousarray(bh.transpose(1, 0, 2).reshape(P, 2 * KH))
    bop = np.ascontiguousarray(b_out.reshape(KV, P).T)        # [P, KV]
    sosp = np.ascontiguousarray(sos.reshape(2, P).T)          # [P, 2]

    shared = {
        "whhp": _pack_k(WS * W_hh.T).astype(F8NP),
        "woutp": _pack_k(WS * W_out.T).astype(F8NP),
        "wihp": _pack_k(WS * W_ih.T)[0].astype(F8NP),         # EMB=256 -> 1 pair
        "wembp": _pack_k(WS * W_emb.T).astype(F8NP),
        "bhp": bhp,
        "bop": bop,
        "sosp": sosp,
    }
    in_maps = []
    for c in range(NCORES):
        sl = slice(c * NS, (c + 1) * NS)
        m = dict(shared)
        m["h0p"] = np.ascontiguousarray(h0p[:, :, :, sl])
        m["gT"] = np.ascontiguousarray(gT[:, :, sl])
        in_maps.append(m)
    return in_maps


def gather_out(results):
    full = np.concatenate([r["uout"] for r in results], axis=2)  # [NOS,V,N] bf16
    u = np.ascontiguousarray(full.transpose(2, 0, 1)).astype(np.float32)
    u /= u.sum(axis=2, keepdims=True)                            # x = u / s
    return u


_NC_CACHE = {}


def get_nc():
    if "nc" not in _NC_CACHE:
        _NC_CACHE["nc"] = build_nc()
    return _NC_CACHE["nc"]


def kernel(**inputs) -> np.ndarray:
    nc = get_nc()
    in_maps = make_in_maps(**inputs)
    res = run_bass_kernel_spmd(nc, in_maps, list(range(NCORES)))
    return gather_out(res.results)
